# revision 1
# baseline (speedup 1.0000x reference)
"""GAU (gated attention unit, relu^2 linear attention) Trainium2 kernel.

Sharding: 8 cores = batch (4) x T-half (2).  Each core handles 2048 rows
of both the query and key/value streams of one batch.  The kv/k_sum
reduction over keys is completed with a 2-rank AllReduce between the two
cores of each batch (fp32 payload); everything else is fully local.

Matmuls run in bf16 with fp32 PSUM accumulation; inputs are cast to
bf16 on the host.  All elementwise/bias/normalization math and the kv
cross-core reduction stay fp32.
"""
import sys

sys.path.insert(0, "/opt/trn_rl_repo")

import ml_dtypes
import numpy as np
import concourse.bass as bass
import concourse.mybir as mybir
import concourse.tile as tile
from concourse.bass_utils import run_bass_kernel_spmd

AF = mybir.ActivationFunctionType
ALU = mybir.AluOpType
F32 = mybir.dt.float32
BF16 = mybir.dt.bfloat16
NPBF = ml_dtypes.bfloat16

N_CORES = 8
D = 1024
S = 512
EPS = 1e-6


def split_sync_waits(nc, max_waits=1):
    """The pinned walrus accepts at most one sync wait per instruction;
    hoist excess waits onto same-engine NoOps inserted before the
    offending instruction (same engine => identical semantics)."""
    n = 0
    for bb in nc.main_func.blocks:
        out = []
        for inst in bb.instructions:
            si = inst.sync_info
            if si is not None and si.on_wait and len(si.on_wait) > max_waits:
                waits = list(si.on_wait)
                spill, keep = waits[:-max_waits], waits[-max_waits:]
                for j in range(0, len(spill), max_waits):
                    nop = mybir.InstNoOp(
                        name=f"{inst.name}_wsp{j}",
                        engine=inst.engine,
                        ins=[],
                        outs=[],
                        bass_nofuse=True,
                        sync_info=mybir.SyncInfo(
                            on_wait=spill[j : j + max_waits], on_update=[]
                        ),
                    )
                    nc.register_instruction(nop)
                    out.append(nop)
                    n += 1
                si.on_wait[:] = keep
            out.append(inst)
        bb.instructions[:] = out
    return n


def build_nc(T=2048, use_collective=True):
    NT = T // 128  # t-chunks (phase 1)
    NQ = T // 512  # q-chunks (phase 2)
    ND = D // 128  # contraction chunks over d
    NS = S // 128  # s-tiles
    NF = D // 128  # f-tiles (gate dim)

    nc = bass.Bass("TRN2", target_bir_lowering=False, debug=False,
                   num_devices=N_CORES)

    # ---- I/O ------------------------------------------------------------
    qT = nc.dram_tensor("qT", [D, T], BF16, kind="ExternalInput")
    kT = nc.dram_tensor("kT", [D, T], BF16, kind="ExternalInput")
    vT = nc.dram_tensor("vT", [D, T], BF16, kind="ExternalInput")
    wgT = nc.dram_tensor("wgT", [D, D], BF16, kind="ExternalInput")
    wvT = nc.dram_tensor("wvT", [D, D], BF16, kind="ExternalInput")
    wqT = nc.dram_tensor("wqT", [D, S], BF16, kind="ExternalInput")
    wkT = nc.dram_tensor("wkT", [D, S], BF16, kind="ExternalInput")
    woT = nc.dram_tensor("woT", [D, D], BF16, kind="ExternalInput")
    bgc = nc.dram_tensor("bgc", [D, 1], F32, kind="ExternalInput")
    bqc = nc.dram_tensor("bqc", [S, 1], F32, kind="ExternalInput")
    bkr = nc.dram_tensor("bkr", [1, S], BF16, kind="ExternalInput")
    bvr = nc.dram_tensor("bvr", [1, D], BF16, kind="ExternalInput")
    bor = nc.dram_tensor("bor", [128, D], F32, kind="ExternalInput")
    onr = nc.dram_tensor("onr", [1, 128], BF16, kind="ExternalInput")
    onc = nc.dram_tensor("onc", [128, 2], BF16, kind="ExternalInput")
    km = nc.dram_tensor("km", [T, 1], F32, kind="ExternalInput")
    qm = nc.dram_tensor("qm", [T, 1], F32, kind="ExternalInput")
    out = nc.dram_tensor("out", [T, D], F32, kind="ExternalOutput")

    qT3 = qT.ap().rearrange("(c p) t -> p c t", p=128)
    kT3 = kT.ap().rearrange("(c p) t -> p c t", p=128)
    vT3 = vT.ap().rearrange("(c p) t -> p c t", p=128)

    with tile.TileContext(nc) as tc:
        with tc.tile_pool(name="const", bufs=1) as cp:
            ones_r = cp.tile([1, 128], BF16)
            nc.sync.dma_start(ones_r[:], onr.ap())
            ones_c = cp.tile([128, 2], BF16)
            nc.sync.dma_start(ones_c[:], onc.ap())
            bk_sb = cp.tile([1, S], BF16)
            nc.sync.dma_start(bk_sb[:], bkr.ap())
            bv_sb = cp.tile([1, D], BF16)
            nc.sync.dma_start(bv_sb[:], bvr.ap())
            bo_bc = cp.tile([128, D], F32)
            nc.sync.dma_start(bo_bc[:], bor.ap())
            bg_sb = cp.tile([128, NF], F32)
            nc.sync.dma_start(bg_sb[:], bgc.ap().rearrange("(c p) o -> p (c o)", p=128))
            bq_sb = cp.tile([128, NS], F32)
            nc.sync.dma_start(bq_sb[:], bqc.ap().rearrange("(c p) o -> p (c o)", p=128))
            km_sb = cp.tile([128, NT], F32)
            nc.sync.dma_start(km_sb[:], km.ap().rearrange("(c p) o -> p (c o)", p=128))
            qm_sb = cp.tile([128, NT], F32)
            nc.sync.dma_start(qm_sb[:], qm.ap().rearrange("(c p) o -> p (c o)", p=128))
            wq_sb = cp.tile([128, ND, S], BF16)
            wqT3 = wqT.ap().rearrange("(c p) s -> p c s", p=128)
            # kv result (post collective), lives through phase 2
            kv_sb = cp.tile([128, NS, D], BF16)
            ks_sb = cp.tile([128, 2 * NS], BF16)

            with tc.tile_pool(name="dram", bufs=1, space="DRAM") as dram, \
                 tc.tile_pool(name="pf", bufs=1) as pf:
                bounce_in = dram.tile([S, D + 1], F32)
                bounce_out = dram.tile([S, D + 1], F32)

                def load_qc(qch):
                    qc = pf.tile([128, ND, 512], BF16, name="qc",
                                 tag="qc", bufs=2)
                    nc.sync.dma_start(
                        qc[:], qT3[:, :, qch * 512:(qch + 1) * 512])
                    return qc
                qc_pre = {}

                # ================= phase 1: k features, v proj, kv =======
                with tc.tile_pool(name="p1", bufs=1) as p1, \
                     tc.tile_pool(name="ps1", bufs=1, space="PSUM") as ps1:
                    wkT3 = wkT.ap().rearrange("(c p) s -> p c s", p=128)
                    wvT3 = wvT.ap().rearrange("(c p) s -> p c s", p=128)
                    wk_sb = p1.tile([128, ND, S], BF16)
                    for c in range(ND):
                        nc.sync.dma_start(wk_sb[:, c, :], wkT3[:, c, :])
                    wv_sb = p1.tile([128, ND, D], BF16)
                    k_nat = p1.tile([128, NT, S], BF16)   # relu^2 key feats
                    v_e1 = p1.tile([128, NT, S], BF16)    # v proj, cols 512:

                    kv0 = [ps1.tile([128, S], F32, name=f"kv0_{s}", tag="kv0",
                                    bufs=NS) for s in range(NS)]

                    for t in range(NT):
                        kc = p1.tile([128, ND, 128], BF16, name="kc",
                                     tag="kc", bufs=3)
                        nc.sync.dma_start(kc[:], kT3[:, :, t * 128:(t + 1) * 128])
                        vc = p1.tile([128, ND, 128], BF16, name="vc",
                                     tag="vc", bufs=3)
                        nc.sync.dma_start(vc[:], vT3[:, :, t * 128:(t + 1) * 128])
                        if t == 0:
                            for c in range(ND):
                                nc.sync.dma_start(wv_sb[:, c, :], wvT3[:, c, :])
                        if t == 1:
                            qc_pre[0] = load_qc(0)
                            for c in range(ND):
                                nc.sync.dma_start(wq_sb[:, c, :], wqT3[:, c, :])

                        # k features: relu(K Wk^T + bk)^2 * km -> k_nat[:,t,:]
                        kb = ps1.tile([128, S], F32, name="kb", tag="kb", bufs=2)
                        for c in range(ND):
                            nc.tensor.matmul(kb[:], kc[:, c, :], wk_sb[:, c, :],
                                             start=(c == 0), stop=False)
                        nc.tensor.matmul(kb[:], ones_r[:], bk_sb[:],
                                         start=False, stop=True)
                        krelu = p1.tile([128, S], F32, name="krelu",
                                        tag="krelu", bufs=2)
                        nc.scalar.activation(krelu[:], kb[:], AF.Relu)
                        nc.vector.scalar_tensor_tensor(
                            k_nat[:, t, :], krelu[:], km_sb[:, t:t + 1], krelu[:],
                            op0=ALU.mult, op1=ALU.mult)

                        # v projection: V Wv^T + bv -> [128, 1024]
                        vb = ps1.tile([128, D], F32, name="vb", tag="vb", bufs=1)
                        for half in range(2):
                            for c in range(ND):
                                nc.tensor.matmul(
                                    vb[:, half * S:(half + 1) * S],
                                    vc[:, c, :],
                                    wv_sb[:, c, half * S:(half + 1) * S],
                                    start=(c == 0), stop=False)
                            nc.tensor.matmul(
                                vb[:, half * S:(half + 1) * S], ones_r[:],
                                bv_sb[:, half * S:(half + 1) * S],
                                start=False, stop=True)
                        v0 = p1.tile([128, S], BF16, name="v0", tag="v0", bufs=2)
                        nc.scalar.activation(v0[:], vb[:, 0:S], AF.Copy)
                        nc.scalar.activation(v_e1[:, t, :], vb[:, S:2 * S], AF.Copy)

                        # kv e-half 0 accumulates across the whole t loop
                        for s in range(NS):
                            nc.tensor.matmul(
                                kv0[s][:], k_nat[:, t, s * 128:(s + 1) * 128],
                                v0[:], start=(t == 0), stop=(t == NT - 1))

                    for s in range(NS):
                        kvst = p1.tile([128, S], F32, name="kvst",
                                       tag="kvst", bufs=2)
                        nc.scalar.activation(kvst[:], kv0[s][:], AF.Copy)
                        nc.sync.dma_start(
                            bounce_in[s * 128:(s + 1) * 128, 0:S], kvst[:])

                # kv e-half 1 + k_sum (separate PSUM pool after ps1 frees)
                with tc.tile_pool(name="p1b", bufs=1) as p1b, \
                     tc.tile_pool(name="ps1b", bufs=1, space="PSUM") as ps1b:
                    for s in range(NS):
                        kv1 = ps1b.tile([128, S], F32, name=f"kv1_{s}",
                                        tag="kv1", bufs=2)
                        for t in range(NT):
                            nc.tensor.matmul(
                                kv1[:], k_nat[:, t, s * 128:(s + 1) * 128],
                                v_e1[:, t, :], start=(t == 0), stop=(t == NT - 1))
                        kvst1 = p1b.tile([128, S], F32, name="kvst1",
                                         tag="kvst1", bufs=2)
                        nc.scalar.activation(kvst1[:], kv1[:], AF.Copy)
                        nc.sync.dma_start(
                            bounce_in[s * 128:(s + 1) * 128, S:2 * S], kvst1[:])
                        ks = ps1b.tile([128, 2], F32, name=f"ks_{s}",
                                       tag="ks", bufs=2)
                        for t in range(NT):
                            nc.tensor.matmul(
                                ks[:], k_nat[:, t, s * 128:(s + 1) * 128],
                                ones_c[:], start=(t == 0), stop=(t == NT - 1))
                        ksst = p1b.tile([128, 1], F32, name="ksst",
                                        tag="ksst", bufs=2)
                        nc.scalar.activation(ksst[:], ks[:, 0:1], AF.Copy)
                        nc.sync.dma_start(
                            bounce_in[s * 128:(s + 1) * 128, D:D + 1], ksst[:])

                with tc.tile_pool(name="p2", bufs=1) as p2, \
                     tc.tile_pool(name="ps2", bufs=1, space="PSUM") as ps2:
                    if use_collective:
                        nc.gpsimd.collective_compute(
                            "AllReduce", ALU.add,
                            replica_groups=[[0, 1], [2, 3], [4, 5], [6, 7]],
                            ins=[bounce_in.opt()], outs=[bounce_out.opt()])
                        kv_src = bounce_out
                    else:
                        kv_src = bounce_in

                    wgT3 = wgT.ap().rearrange("(c p) s -> p c s", p=128)
                    wg_sb = p2.tile([128, ND, D], BF16)
                    for c in range(ND):
                        nc.sync.dma_start(wg_sb[:, c, :], wgT3[:, c, :])
                    woT3 = woT.ap().rearrange("(c p) s -> p c s", p=128)
                    wo_sb = p2.tile([128, ND, D], BF16)
                    for c in range(ND):
                        nc.sync.dma_start(wo_sb[:, c, :], woT3[:, c, :])

                    q_sb = p2.tile([128, NS, T], BF16)
                    u_sb = p2.tile([128, NF, T], BF16)

                    # ---- pass A: q features + u gate (no kv dependency) --
                    for qch in range(NQ):
                        t0 = qch * 512
                        qc = qc_pre.pop(qch, None)
                        if qc is None:
                            qc = load_qc(qch)
                        for s in range(NS):
                            qf = ps2.tile([128, 512], F32, name="qf",
                                          tag="mm", bufs=3)
                            for c in range(ND):
                                nc.tensor.matmul(
                                    qf[:], wq_sb[:, c, s * 128:(s + 1) * 128],
                                    qc[:, c, :], start=(c == 0), stop=(c == ND - 1))
                            qrelu = p2.tile([128, 512], F32, name="qrelu",
                                            tag="qrelu", bufs=2)
                            nc.scalar.activation(qrelu[:], qf[:], AF.Relu,
                                                 bias=bq_sb[:, s:s + 1])
                            nc.vector.tensor_mul(q_sb[:, s, t0:t0 + 512],
                                                 qrelu[:], qrelu[:])
                        for f in range(NF):
                            uf = ps2.tile([128, 512], F32, name="uf",
                                          tag="mm", bufs=3)
                            for c in range(ND):
                                nc.tensor.matmul(
                                    uf[:], wg_sb[:, c, f * 128:(f + 1) * 128],
                                    qc[:, c, :], start=(c == 0), stop=(c == ND - 1))
                            nc.scalar.activation(u_sb[:, f, t0:t0 + 512], uf[:],
                                                 AF.Silu, bias=bg_sb[:, f:f + 1])

                    # ---- unpack kv + k_sum (after pass A in engine order) -
                    for c in range(NS):
                        kv_f = p2.tile([128, D], F32, name="kv_f",
                                       tag="kv_f", bufs=2)
                        nc.sync.dma_start(
                            kv_f[:], kv_src[c * 128:(c + 1) * 128, 0:D])
                        nc.vector.tensor_copy(kv_sb[:, c, :], kv_f[:])
                    ks_f = p2.tile([128, NS], F32)
                    nc.sync.dma_start(
                        ks_f[:],
                        kv_src[:, D:D + 1].rearrange("(c p) o -> p (c o)", p=128))
                    for c in range(NS):
                        for j in range(2):
                            nc.vector.tensor_copy(
                                ks_sb[:, 2 * c + j:2 * c + j + 1],
                                ks_f[:, c:c + 1])

                    # ---- pass B: qkv, z, gated output projection ---------
                    for qch in range(NQ):
                        t0 = qch * 512
                        for f in range(NF):
                            qk = ps2.tile([128, 512], F32, name="qk",
                                          tag="mm", bufs=3)
                            for c in range(NS):
                                nc.tensor.matmul(
                                    qk[:], kv_sb[:, c, f * 128:(f + 1) * 128],
                                    q_sb[:, c, t0:t0 + 512],
                                    start=(c == 0), stop=(c == NS - 1))
                            nc.vector.tensor_mul(u_sb[:, f, t0:t0 + 512], qk[:],
                                                 u_sb[:, f, t0:t0 + 512])
                        for tt in range(4):
                            ti = qch * 4 + tt
                            zp = ps2.tile([128, 2], F32, name="zp",
                                          tag="z", bufs=2)
                            for c in range(NS):
                                nc.tensor.matmul(
                                    zp[:],
                                    q_sb[:, c, ti * 128:(ti + 1) * 128],
                                    ks_sb[:, 2 * c:2 * c + 2], start=(c == 0),
                                    stop=(c == NS - 1))
                            z_sb = p2.tile([128, 1], F32, name="z_sb",
                                           tag="z_sb", bufs=2)
                            nc.vector.tensor_scalar_add(z_sb[:], zp[:, 0:1], EPS)
                            zi = p2.tile([128, 1], F32, name="zi",
                                         tag="zi", bufs=2)
                            nc.vector.reciprocal(zi[:], z_sb[:])
                            nc.vector.tensor_mul(zi[:], zi[:], qm_sb[:, ti:ti + 1])

                            o_sb = p2.tile([128, D], F32, name="o_sb",
                                           tag="o_sb", bufs=2)
                            for half in range(2):
                                op = ps2.tile([128, 512], F32, name="op",
                                              tag="out", bufs=2)
                                for f in range(NF):
                                    nc.tensor.matmul(
                                        op[:],
                                        u_sb[:, f, ti * 128:(ti + 1) * 128],
                                        wo_sb[:, f, half * S:(half + 1) * S],
                                        start=(f == 0), stop=(f == NF - 1))
                                nc.scalar.activation(
                                    o_sb[:, half * S:(half + 1) * S], op[:],
                                    AF.Copy, scale=zi[:])
                                nc.vector.scalar_tensor_tensor(
                                    o_sb[:, half * S:(half + 1) * S],
                                    bo_bc[:, half * S:(half + 1) * S],
                                    qm_sb[:, ti:ti + 1],
                                    o_sb[:, half * S:(half + 1) * S],
                                    op0=ALU.mult, op1=ALU.add)
                            nc.sync.dma_start(
                                out.ap()[ti * 128:(ti + 1) * 128, :], o_sb[:])

    split_sync_waits(nc)
    return nc


_NC_CACHE = {}


def _get_nc(T, use_collective=True):
    key = (T, use_collective)
    if key not in _NC_CACHE:
        _NC_CACHE[key] = build_nc(T, use_collective)
    return _NC_CACHE[key]


def make_in_maps(queries, keys, values, query_mask, key_mask,
                 Wg, bg, Wv, bv, Wq, bq, Wk, bk, Wo, bo):
    B, T_full, _ = queries.shape
    Th = T_full // 2
    f32 = np.float32
    qTb = np.ascontiguousarray(queries.transpose(0, 2, 1)).astype(NPBF)
    kTb = np.ascontiguousarray(keys.transpose(0, 2, 1)).astype(NPBF)
    vTb = np.ascontiguousarray(values.transpose(0, 2, 1)).astype(NPBF)
    shared = {
        "wgT": np.ascontiguousarray(Wg.T).astype(NPBF),
        "wvT": np.ascontiguousarray(Wv.T).astype(NPBF),
        "wqT": np.ascontiguousarray(Wq.T).astype(NPBF),
        "wkT": np.ascontiguousarray(Wk.T).astype(NPBF),
        "woT": np.ascontiguousarray(Wo.T).astype(NPBF),
        "bgc": np.asarray(bg, f32).reshape(D, 1),
        "bqc": np.asarray(bq, f32).reshape(S, 1),
        "bkr": np.asarray(bk, f32).reshape(1, S).astype(NPBF),
        "bvr": np.asarray(bv, f32).reshape(1, D).astype(NPBF),
        "bor": np.tile(np.asarray(bo, f32).reshape(1, D), (128, 1)),
        "onr": np.ones((1, 128), NPBF),
        "onc": np.ones((128, 2), NPBF),
    }
    in_maps = []
    for c in range(N_CORES):
        b, h = divmod(c, 2)
        sl = slice(h * Th, (h + 1) * Th)
        m = dict(shared)
        m["qT"] = np.ascontiguousarray(qTb[b][:, sl])
        m["kT"] = np.ascontiguousarray(kTb[b][:, sl])
        m["vT"] = np.ascontiguousarray(vTb[b][:, sl])
        m["km"] = np.asarray(key_mask[b, sl], f32).reshape(Th, 1)
        m["qm"] = np.asarray(query_mask[b, sl], f32).reshape(Th, 1)
        in_maps.append(m)
    return in_maps


def kernel(queries, keys, values, query_mask, key_mask,
           Wg, bg, Wv, bv, Wq, bq, Wk, bk, Wo, bo, _trace=False):
    B, T_full, _ = queries.shape
    Th = T_full // 2
    nc = _get_nc(Th)
    in_maps = make_in_maps(queries, keys, values, query_mask, key_mask,
                           Wg, bg, Wv, bv, Wq, bq, Wk, bk, Wo, bo)
    res = run_bass_kernel_spmd(nc, in_maps, core_ids=list(range(N_CORES)),
                               trace=_trace)
    out = np.empty((B, T_full, D), np.float32)
    for c in range(N_CORES):
        b, h = divmod(c, 2)
        out[b, h * Th:(h + 1) * Th] = res.results[c]["out"]
    if _trace:
        kernel._last_res = res
    return out



# revision 10
# speedup vs baseline: 1.2566x; 1.2566x over previous
"""GAU (gated attention unit, relu^2 linear attention) Trainium2 kernel.

Sharding: 8 cores = batch (4) x T-half (2).  Each core handles 2048 rows
of both the query and key/value streams of one batch.  The kv/k_sum
reduction over keys is completed with a 2-rank AllReduce between the two
cores of each batch (bf16 payload); everything else is fully local.

Matmuls run in fp8(e4m3) with DoubleRow perf mode (2 contraction
subtiles per instruction, 0.5 cycles/row) except the output projection,
which stays bf16 for accuracy.  PSUM accumulation is fp32 throughout;
scales keep every fp8 operand inside e4m3 range:
  kv stored as kv/32, k_sum stored as k_sum/8, undone via the final
  per-token 1/z scale.
"""
import sys

sys.path.insert(0, "/opt/trn_rl_repo")

import ml_dtypes
import numpy as np
import concourse.bass as bass
import concourse.mybir as mybir
import concourse.tile as tile
from concourse.bass_utils import run_bass_kernel_spmd

AF = mybir.ActivationFunctionType
ALU = mybir.AluOpType
PM = mybir.MatmulPerfMode
F32 = mybir.dt.float32
BF16 = mybir.dt.bfloat16
F8 = mybir.dt.float8e4
NPBF = ml_dtypes.bfloat16
NPF8 = mybir.dt.np(F8)

N_CORES = 8
D = 1024
S = 512
EPS = 1e-6
KV_SCL = 32.0   # kv_sb holds kv/32
KS_SCL = 8.0    # ks_sb holds k_sum/8


def split_sync_waits(nc, max_waits=1):
    """The pinned walrus accepts at most one sync wait per instruction;
    hoist excess waits onto same-engine NoOps inserted before the
    offending instruction (same engine => identical semantics)."""
    n = 0
    for bb in nc.main_func.blocks:
        out = []
        for inst in bb.instructions:
            si = inst.sync_info
            if si is not None and si.on_wait and len(si.on_wait) > max_waits:
                waits = list(si.on_wait)
                spill, keep = waits[:-max_waits], waits[-max_waits:]
                for j in range(0, len(spill), max_waits):
                    nop = mybir.InstNoOp(
                        name=f"{inst.name}_wsp{j}",
                        engine=inst.engine,
                        ins=[],
                        outs=[],
                        bass_nofuse=True,
                        sync_info=mybir.SyncInfo(
                            on_wait=spill[j : j + max_waits], on_update=[]
                        ),
                    )
                    nc.register_instruction(nop)
                    out.append(nop)
                    n += 1
                si.on_wait[:] = keep
            out.append(inst)
        bb.instructions[:] = out
    return n


def build_nc(T=2048, use_collective=True):
    NT = T // 128   # t-chunks (phase 1)
    NQ = T // 512   # q-chunks (phase 2)
    ND = D // 128   # contraction chunks over d
    ND2 = ND // 2   # DoubleRow pairs over d
    NS = S // 128   # s-tiles
    NS2 = NS // 2
    NF = D // 128   # f-tiles (gate dim)

    nc = bass.Bass("TRN2", target_bir_lowering=False, debug=False,
                   num_devices=N_CORES)

    # ---- I/O ------------------------------------------------------------
    # inputs pre-chunked on host: one dense DMA per tile
    qTc = nc.dram_tensor("qTc", [NQ, 128, ND * 512], F8, kind="ExternalInput")
    kTc = nc.dram_tensor("kTc", [NT, 128, ND * 128], F8, kind="ExternalInput")
    vTc = nc.dram_tensor("vTc", [NT, 128, ND * 128], F8, kind="ExternalInput")
    wgc = nc.dram_tensor("wgc", [128, ND * D], F8, kind="ExternalInput")
    wvc = nc.dram_tensor("wvc", [128, ND * D], F8, kind="ExternalInput")
    wqc = nc.dram_tensor("wqc", [128, ND * S], F8, kind="ExternalInput")
    wkc = nc.dram_tensor("wkc", [128, ND * S], F8, kind="ExternalInput")
    woc = nc.dram_tensor("woc", [128, ND * D], BF16, kind="ExternalInput")
    bgc = nc.dram_tensor("bgc", [128, NF], F32, kind="ExternalInput")
    bqc = nc.dram_tensor("bqc", [128, NS], F32, kind="ExternalInput")
    bkr = nc.dram_tensor("bkr", [1, S], BF16, kind="ExternalInput")
    bvr = nc.dram_tensor("bvr", [1, D], BF16, kind="ExternalInput")
    bor = nc.dram_tensor("bor", [128, D], F32, kind="ExternalInput")
    onr = nc.dram_tensor("onr", [1, 128], BF16, kind="ExternalInput")
    on2 = nc.dram_tensor("on2", [128, 4], F8, kind="ExternalInput")
    km = nc.dram_tensor("km", [128, NT], F32, kind="ExternalInput")
    qm = nc.dram_tensor("qm", [128, NT], F32, kind="ExternalInput")
    out = nc.dram_tensor("out", [T, D], F32, kind="ExternalOutput")

    with tile.TileContext(nc) as tc:
        with tc.tile_pool(name="const", bufs=1) as cp:
            # first-needed tensors first: phase-1 kfeat path
            wk_sb = cp.tile([128, ND, S], F8)
            nc.sync.dma_start(wk_sb[:], wkc.ap())
            ones_r = cp.tile([1, 128], BF16)
            nc.sync.dma_start(ones_r[:], onr.ap())
            bk_sb = cp.tile([1, S], BF16)
            nc.sync.dma_start(bk_sb[:], bkr.ap())
            km_sb = cp.tile([128, NT], F32)
            nc.sync.dma_start(km_sb[:], km.ap())
            bv_sb = cp.tile([1, D], BF16)
            nc.sync.dma_start(bv_sb[:], bvr.ap())
            ones2 = cp.tile([128, 2, 2], F8)
            nc.sync.dma_start(ones2[:], on2.ap())
            qm_sb = cp.tile([128, NT], F32)
            nc.sync.dma_start(qm_sb[:], qm.ap())
            bq_sb = cp.tile([128, NS], F32)
            nc.sync.dma_start(bq_sb[:], bqc.ap())
            bg_sb = cp.tile([128, NF], F32)
            nc.sync.dma_start(bg_sb[:], bgc.ap())
            bo_bc = cp.tile([128, D], F32)
            nc.sync.dma_start(bo_bc[:], bor.ap())
            wq_sb = cp.tile([128, ND, S], F8)
            # persistent across phases
            k_nat = cp.tile([128, NT, S], F8)    # relu^2 key feats
            v0_all = cp.tile([128, NT, S], F8)   # v proj, cols :512
            v_e1 = cp.tile([128, NT, S], F8)     # v proj, cols 512:
            kv_sb = cp.tile([128, NS, D], F8)    # kv/32 (post collective)
            ks_sb = cp.tile([128, NS, 2], F8)    # k_sum/8, duplicated cols
            q_sb = cp.tile([128, NS, T], F8)
            u_sb = cp.tile([128, NF, T], BF16)

            with tc.tile_pool(name="dram", bufs=1, space="DRAM") as dram, \
                 tc.tile_pool(name="pf", bufs=1) as pf:
                bounce_in = dram.tile([S, D + 1], BF16)
                bounce_out = dram.tile([S, D + 1], BF16)

                def load_qc(qch):
                    qc = pf.tile([128, ND, 512], F8, name="qc",
                                 tag="qc", bufs=3)
                    nc.sync.dma_start(qc[:], qTc.ap()[qch])
                    return qc
                qc_pre = {}

                # ================= phase 1: k features, v proj, kv =======
                with tc.tile_pool(name="p1", bufs=1) as p1, \
                     tc.tile_pool(name="ps1", bufs=1, space="PSUM") as ps1:
                    wv_sb = p1.tile([128, ND, D], F8)

                    kv0 = [ps1.tile([128, S], F32, name=f"kv0_{s}", tag="kv0",
                                    bufs=NS) for s in range(NS)]

                    for t in range(NT):
                        kc = p1.tile([128, ND, 128], F8, name="kc",
                                     tag="kc", bufs=3)
                        nc.sync.dma_start(kc[:], kTc.ap()[t])
                        vc = p1.tile([128, ND, 128], F8, name="vc",
                                     tag="vc", bufs=3)
                        nc.sync.dma_start(vc[:], vTc.ap()[t])
                        if t == 0:
                            nc.sync.dma_start(wv_sb[:], wvc.ap())
                        if t == 1:
                            qc_pre[0] = load_qc(0)
                            nc.sync.dma_start(wq_sb[:], wqc.ap())

                        # k features: relu(K Wk^T + bk)^2 * km -> k_nat[:,t,:]
                        kb = ps1.tile([128, S], F32, name="kb", tag="kb", bufs=2)
                        for c in range(ND2):
                            nc.tensor.matmul(kb[:], kc[:, 2 * c:2 * c + 2, :],
                                             wk_sb[:, 2 * c:2 * c + 2, :],
                                             start=(c == 0), stop=False,
                                             perf_mode=PM.DoubleRow)
                        nc.tensor.matmul(kb[:], ones_r[:], bk_sb[:],
                                         start=False, stop=True)
                        krelu = p1.tile([128, S], F32, name="krelu",
                                        tag="krelu", bufs=2)
                        nc.scalar.activation(krelu[:], kb[:], AF.Relu)
                        nc.vector.scalar_tensor_tensor(
                            k_nat[:, t, :], krelu[:], km_sb[:, t:t + 1], krelu[:],
                            op0=ALU.mult, op1=ALU.mult)

                        # v projection: V Wv^T + bv -> [128, 1024]
                        vb = ps1.tile([128, D], F32, name="vb", tag="vb", bufs=1)
                        for half in range(2):
                            for c in range(ND2):
                                nc.tensor.matmul(
                                    vb[:, half * S:(half + 1) * S],
                                    vc[:, 2 * c:2 * c + 2, :],
                                    wv_sb[:, 2 * c:2 * c + 2,
                                          half * S:(half + 1) * S],
                                    start=(c == 0), stop=False,
                                    perf_mode=PM.DoubleRow)
                            nc.tensor.matmul(
                                vb[:, half * S:(half + 1) * S], ones_r[:],
                                bv_sb[:, half * S:(half + 1) * S],
                                start=False, stop=True)
                        nc.vector.tensor_copy(v0_all[:, t, :], vb[:, 0:S])
                        nc.vector.tensor_copy(v_e1[:, t, :], vb[:, S:2 * S])

                        # kv e-half 0 accumulates across t pairs (DoubleRow)
                        if t % 2 == 1:
                            for s in range(NS):
                                nc.tensor.matmul(
                                    kv0[s][:],
                                    k_nat[:, t - 1:t + 1, s * 128:(s + 1) * 128],
                                    v0_all[:, t - 1:t + 1, :],
                                    start=(t == 1), stop=(t == NT - 1),
                                    perf_mode=PM.DoubleRow)

                    for s in range(NS):
                        kvst = p1.tile([128, S], BF16, name="kvst",
                                       tag="kvst", bufs=2)
                        nc.scalar.activation(kvst[:], kv0[s][:], AF.Copy)
                        nc.sync.dma_start(
                            bounce_in[s * 128:(s + 1) * 128, 0:S], kvst[:])

                # kv e-half 1 + k_sum (separate PSUM pool after ps1 frees)
                with tc.tile_pool(name="p1b", bufs=1) as p1b, \
                     tc.tile_pool(name="ps1b", bufs=1, space="PSUM") as ps1b:
                    for s in range(NS):
                        kv1 = ps1b.tile([128, S], F32, name=f"kv1_{s}",
                                        tag="kv1", bufs=2)
                        for tp in range(NT // 2):
                            nc.tensor.matmul(
                                kv1[:],
                                k_nat[:, 2 * tp:2 * tp + 2,
                                      s * 128:(s + 1) * 128],
                                v_e1[:, 2 * tp:2 * tp + 2, :],
                                start=(tp == 0), stop=(tp == NT // 2 - 1),
                                perf_mode=PM.DoubleRow)
                        kvst1 = p1b.tile([128, S], BF16, name="kvst1",
                                         tag="kvst1", bufs=2)
                        nc.scalar.activation(kvst1[:], kv1[:], AF.Copy)
                        nc.sync.dma_start(
                            bounce_in[s * 128:(s + 1) * 128, S:2 * S], kvst1[:])
                        ks = ps1b.tile([128, 2], F32, name=f"ks_{s}",
                                       tag="ks", bufs=2)
                        for t in range(NT):
                            nc.tensor.matmul(
                                ks[:],
                                k_nat[:, t, s * 128:(s + 1) * 128],
                                ones2[:, 0, :], start=(t == 0),
                                stop=(t == NT - 1))
                        ksst = p1b.tile([128, 1], BF16, name="ksst",
                                        tag="ksst", bufs=2)
                        nc.scalar.activation(ksst[:], ks[:, 0:1], AF.Copy)
                        nc.sync.dma_start(
                            bounce_in[s * 128:(s + 1) * 128, D:D + 1], ksst[:])

                with tc.tile_pool(name="p2", bufs=1) as p2, \
                     tc.tile_pool(name="ps2", bufs=1, space="PSUM") as ps2:
                    if use_collective:
                        nc.gpsimd.collective_compute(
                            "AllReduce", ALU.add,
                            replica_groups=[[0, 1], [2, 3], [4, 5], [6, 7]],
                            ins=[bounce_in.opt()], outs=[bounce_out.opt()])
                        kv_src = bounce_out
                    else:
                        kv_src = bounce_in

                    wg_sb = p2.tile([128, ND, D], F8)
                    nc.sync.dma_start(wg_sb[:], wgc.ap())
                    wo_sb = p2.tile([128, ND, D], BF16)
                    nc.sync.dma_start(wo_sb[:], woc.ap())

                    # ---- pass A: q features + u gate (no kv dependency) --
                    for qch in range(NQ):
                        t0 = qch * 512
                        qc = qc_pre.pop(qch, None)
                        if qc is None:
                            qc = load_qc(qch)
                        if qch + 1 < NQ and (qch + 1) not in qc_pre:
                            qc_pre[qch + 1] = load_qc(qch + 1)
                        for s in range(NS):
                            qf = ps2.tile([128, 512], F32, name="qf",
                                          tag="mm", bufs=3)
                            for c in range(ND2):
                                nc.tensor.matmul(
                                    qf[:],
                                    wq_sb[:, 2 * c:2 * c + 2,
                                          s * 128:(s + 1) * 128],
                                    qc[:, 2 * c:2 * c + 2, :],
                                    start=(c == 0), stop=(c == ND2 - 1),
                                    perf_mode=PM.DoubleRow)
                            qrelu = p2.tile([128, 512], BF16, name="qrelu",
                                            tag="qrelu", bufs=2)
                            nc.scalar.activation(qrelu[:], qf[:], AF.Relu,
                                                 bias=bq_sb[:, s:s + 1])
                            nc.vector.tensor_mul(q_sb[:, s, t0:t0 + 512],
                                                 qrelu[:], qrelu[:])
                        for f in range(NF):
                            uf = ps2.tile([128, 512], F32, name="uf",
                                          tag="mm", bufs=3)
                            for c in range(ND2):
                                nc.tensor.matmul(
                                    uf[:],
                                    wg_sb[:, 2 * c:2 * c + 2,
                                          f * 128:(f + 1) * 128],
                                    qc[:, 2 * c:2 * c + 2, :],
                                    start=(c == 0), stop=(c == ND2 - 1),
                                    perf_mode=PM.DoubleRow)
                            nc.scalar.activation(u_sb[:, f, t0:t0 + 512], uf[:],
                                                 AF.Silu, bias=bg_sb[:, f:f + 1])

                    # ---- unpack kv + k_sum (after pass A in engine order) -
                    for c in range(NS):
                        kv_f = p2.tile([128, D], BF16, name="kv_f",
                                       tag="kv_f", bufs=2)
                        nc.sync.dma_start(
                            kv_f[:], kv_src[c * 128:(c + 1) * 128, 0:D])
                        nc.gpsimd.tensor_scalar_mul(kv_sb[:, c, :], kv_f[:],
                                                    1.0 / KV_SCL)
                    ks_f = p2.tile([128, NS], BF16)
                    nc.sync.dma_start(
                        ks_f[:],
                        kv_src[:, D:D + 1].rearrange("(c p) o -> p (c o)", p=128))
                    for c in range(NS):
                        for j in range(2):
                            nc.gpsimd.tensor_scalar_mul(
                                ks_sb[:, c, j:j + 1], ks_f[:, c:c + 1],
                                1.0 / KS_SCL)

                    # ---- pass B: qkv, z, gated output projection ---------
                    for qch in range(NQ):
                        t0 = qch * 512
                        for f in range(NF):
                            qk = ps2.tile([128, 512], F32, name="qk",
                                          tag="mm", bufs=3)
                            for c in range(NS2):
                                nc.tensor.matmul(
                                    qk[:],
                                    kv_sb[:, 2 * c:2 * c + 2,
                                          f * 128:(f + 1) * 128],
                                    q_sb[:, 2 * c:2 * c + 2, t0:t0 + 512],
                                    start=(c == 0), stop=(c == NS2 - 1),
                                    perf_mode=PM.DoubleRow)
                            nc.vector.tensor_mul(u_sb[:, f, t0:t0 + 512], qk[:],
                                                 u_sb[:, f, t0:t0 + 512])
                        for tt in range(4):
                            ti = qch * 4 + tt
                            zp = ps2.tile([128, 2], F32, name="zp",
                                          tag="z", bufs=2)
                            for c in range(NS):
                                nc.tensor.matmul(
                                    zp[:],
                                    q_sb[:, c, ti * 128:(ti + 1) * 128],
                                    ks_sb[:, c, :],
                                    start=(c == 0), stop=(c == NS - 1))
                            z_sb = p2.tile([128, 1], F32, name="z_sb",
                                           tag="z_sb", bufs=2)
                            nc.vector.tensor_scalar_add(z_sb[:], zp[:, 0:1],
                                                        EPS / KS_SCL)
                            zi = p2.tile([128, 1], F32, name="zi",
                                         tag="zi", bufs=2)
                            nc.vector.reciprocal(zi[:], z_sb[:])
                            # zi *= qm * (KV_SCL/KS_SCL)
                            nc.vector.scalar_tensor_tensor(
                                zi[:], qm_sb[:, ti:ti + 1], KV_SCL / KS_SCL,
                                zi[:], op0=ALU.mult, op1=ALU.mult)

                            o_sb = p2.tile([128, D], F32, name="o_sb",
                                           tag="o_sb", bufs=2)
                            for half in range(2):
                                op = ps2.tile([128, 512], F32, name="op",
                                              tag="out", bufs=2)
                                for f in range(NF):
                                    nc.tensor.matmul(
                                        op[:],
                                        u_sb[:, f, ti * 128:(ti + 1) * 128],
                                        wo_sb[:, f, half * S:(half + 1) * S],
                                        start=(f == 0), stop=(f == NF - 1))
                                nc.scalar.activation(
                                    o_sb[:, half * S:(half + 1) * S], op[:],
                                    AF.Copy, scale=zi[:])
                                nc.vector.scalar_tensor_tensor(
                                    o_sb[:, half * S:(half + 1) * S],
                                    bo_bc[:, half * S:(half + 1) * S],
                                    qm_sb[:, ti:ti + 1],
                                    o_sb[:, half * S:(half + 1) * S],
                                    op0=ALU.mult, op1=ALU.add)
                            nc.sync.dma_start(
                                out.ap()[ti * 128:(ti + 1) * 128, :], o_sb[:])

    split_sync_waits(nc)
    return nc


_NC_CACHE = {}


def _get_nc(T, use_collective=True):
    key = (T, use_collective)
    if key not in _NC_CACHE:
        _NC_CACHE[key] = build_nc(T, use_collective)
    return _NC_CACHE[key]


def _chunk_T(xT, chunk):
    # [D, T] -> [T//chunk, 128, ND*chunk]: (t, p, c*chunk+j) = x[c*128+p, t*chunk+j]
    Dd, T = xT.shape
    x = xT.reshape(Dd // 128, 128, T // chunk, chunk)
    return np.ascontiguousarray(x.transpose(2, 1, 0, 3).reshape(
        T // chunk, 128, (Dd // 128) * chunk))


def _chunk_W(wT):
    # [D, F] -> [128, ND*F]: (p, c*F+s) = wT[c*128+p, s]
    Dd, Fd = wT.shape
    return np.ascontiguousarray(
        wT.reshape(Dd // 128, 128, Fd).transpose(1, 0, 2).reshape(
            128, (Dd // 128) * Fd))


def make_in_maps(queries, keys, values, query_mask, key_mask,
                 Wg, bg, Wv, bv, Wq, bq, Wk, bk, Wo, bo):
    B, T_full, _ = queries.shape
    Th = T_full // 2
    NT = Th // 128
    f32 = np.float32
    qTb = np.ascontiguousarray(queries.transpose(0, 2, 1)).astype(NPF8)
    kTb = np.ascontiguousarray(keys.transpose(0, 2, 1)).astype(NPF8)
    vTb = np.ascontiguousarray(values.transpose(0, 2, 1)).astype(NPF8)
    shared = {
        "wgc": _chunk_W(np.ascontiguousarray(Wg.T).astype(NPF8)),
        "wvc": _chunk_W(np.ascontiguousarray(Wv.T).astype(NPF8)),
        "wqc": _chunk_W(np.ascontiguousarray(Wq.T).astype(NPF8)),
        "wkc": _chunk_W(np.ascontiguousarray(Wk.T).astype(NPF8)),
        "woc": _chunk_W(np.ascontiguousarray(Wo.T).astype(NPBF)),
        "bgc": np.ascontiguousarray(
            np.asarray(bg, f32).reshape(D // 128, 128).T),
        "bqc": np.ascontiguousarray(
            np.asarray(bq, f32).reshape(S // 128, 128).T),
        "bkr": np.asarray(bk, f32).reshape(1, S).astype(NPBF),
        "bvr": np.asarray(bv, f32).reshape(1, D).astype(NPBF),
        "bor": np.tile(np.asarray(bo, f32).reshape(1, D), (128, 1)),
        "onr": np.ones((1, 128), NPBF),
        "on2": np.ones((128, 4), NPF8),
    }
    in_maps = []
    for c in range(N_CORES):
        b, h = divmod(c, 2)
        sl = slice(h * Th, (h + 1) * Th)
        m = dict(shared)
        m["qTc"] = _chunk_T(qTb[b][:, sl], 512)
        m["kTc"] = _chunk_T(kTb[b][:, sl], 128)
        m["vTc"] = _chunk_T(vTb[b][:, sl], 128)
        m["km"] = np.ascontiguousarray(
            np.asarray(key_mask[b, sl], f32).reshape(NT, 128).T)
        m["qm"] = np.ascontiguousarray(
            np.asarray(query_mask[b, sl], f32).reshape(NT, 128).T)
        in_maps.append(m)
    return in_maps


def kernel(queries, keys, values, query_mask, key_mask,
           Wg, bg, Wv, bv, Wq, bq, Wk, bk, Wo, bo, _trace=False):
    B, T_full, _ = queries.shape
    Th = T_full // 2
    nc = _get_nc(Th)
    in_maps = make_in_maps(queries, keys, values, query_mask, key_mask,
                           Wg, bg, Wv, bv, Wq, bq, Wk, bk, Wo, bo)
    res = run_bass_kernel_spmd(nc, in_maps, core_ids=list(range(N_CORES)),
                               trace=_trace)
    out = np.empty((B, T_full, D), np.float32)
    for c in range(N_CORES):
        b, h = divmod(c, 2)
        out[b, h * Th:(h + 1) * Th] = res.results[c]["out"]
    if _trace:
        kernel._last_res = res
    return out


# revision 21
# speedup vs baseline: 1.6527x; 1.3152x over previous
"""GAU (gated attention unit, relu^2 linear attention) Trainium2 kernel.

Sharding: 8 cores = batch (4) x T-half (2).  Each core handles 2048 rows
of both the query and key/value streams of one batch.  The kv/k_sum
reduction over keys is completed with a 2-rank AllReduce between the two
cores of each batch (bf16 payload); everything else is fully local.

Matmuls run in fp8(e4m3) with DoubleRow perf mode (2 contraction
subtiles per instruction, 0.5 cycles/row) except the output projection,
which stays bf16 for accuracy.  PSUM accumulation is fp32 throughout;
scales keep every fp8 operand inside e4m3 range:
  kv stored as kv/32, k_sum stored as k_sum/8, undone via the final
  per-token 1/z scale.
"""
import sys

sys.path.insert(0, "/opt/trn_rl_repo")

import ml_dtypes
import numpy as np
import concourse.bass as bass
import concourse.mybir as mybir
import concourse.tile as tile
from concourse.bass_utils import run_bass_kernel_spmd

AF = mybir.ActivationFunctionType
ALU = mybir.AluOpType
PM = mybir.MatmulPerfMode
F32 = mybir.dt.float32
BF16 = mybir.dt.bfloat16
F8 = mybir.dt.float8e4
NPBF = ml_dtypes.bfloat16
NPF8 = mybir.dt.np(F8)

N_CORES = 8
D = 1024
S = 512
EPS = 1e-6
KV_SCL = 32.0   # kv_sb holds kv/32
KS_SCL = 8.0    # ks_sb holds k_sum/8


def split_sync_waits(nc, max_waits=1):
    """The pinned walrus accepts at most one sync wait per instruction;
    hoist excess waits onto same-engine NoOps inserted before the
    offending instruction (same engine => identical semantics)."""
    n = 0
    for bb in nc.main_func.blocks:
        out = []
        for inst in bb.instructions:
            si = inst.sync_info
            if si is not None and si.on_wait and len(si.on_wait) > max_waits:
                waits = list(si.on_wait)
                spill, keep = waits[:-max_waits], waits[-max_waits:]
                for j in range(0, len(spill), max_waits):
                    nop = mybir.InstNoOp(
                        name=f"{inst.name}_wsp{j}",
                        engine=inst.engine,
                        ins=[],
                        outs=[],
                        bass_nofuse=True,
                        sync_info=mybir.SyncInfo(
                            on_wait=spill[j : j + max_waits], on_update=[]
                        ),
                    )
                    nc.register_instruction(nop)
                    out.append(nop)
                    n += 1
                si.on_wait[:] = keep
            out.append(inst)
        bb.instructions[:] = out
    return n


def build_nc(T=2048, use_collective=True):
    NT = T // 128   # t-chunks (phase 1)
    NQ = T // 512   # q-chunks (phase 2)
    ND = D // 128   # contraction chunks over d
    ND2 = ND // 2   # DoubleRow pairs over d
    NS = S // 128   # s-tiles
    NS2 = NS // 2
    NF = D // 128   # f-tiles (gate dim)

    nc = bass.Bass("TRN2", target_bir_lowering=False, debug=False,
                   num_devices=N_CORES)

    # ---- I/O ------------------------------------------------------------
    # inputs pre-chunked on host: one dense DMA per tile
    qTc = nc.dram_tensor("qTc", [NQ, 128, ND * 512], F8, kind="ExternalInput")
    kTc = nc.dram_tensor("kTc", [NT, 128, ND * 128], F8, kind="ExternalInput")
    vTc = nc.dram_tensor("vTc", [NT, 128, ND * 128], F8, kind="ExternalInput")
    wgc = nc.dram_tensor("wgc", [128, ND * D], F8, kind="ExternalInput")
    wvc = nc.dram_tensor("wvc", [128, ND * D], F8, kind="ExternalInput")
    wqc = nc.dram_tensor("wqc", [128, ND * S], F8, kind="ExternalInput")
    wkc = nc.dram_tensor("wkc", [128, ND * S], F8, kind="ExternalInput")
    woc = nc.dram_tensor("woc", [128, ND * D], F8, kind="ExternalInput")
    bgc = nc.dram_tensor("bgc", [128, NF], F32, kind="ExternalInput")
    bqc = nc.dram_tensor("bqc", [128, NS], F32, kind="ExternalInput")
    bkr = nc.dram_tensor("bkr", [1, S], BF16, kind="ExternalInput")
    bvr = nc.dram_tensor("bvr", [1, D], BF16, kind="ExternalInput")
    bor = nc.dram_tensor("bor", [128, D], F32, kind="ExternalInput")
    onr = nc.dram_tensor("onr", [1, 128], BF16, kind="ExternalInput")
    on2 = nc.dram_tensor("on2", [128, 4], F8, kind="ExternalInput")
    km = nc.dram_tensor("km", [128, NT], F32, kind="ExternalInput")
    qm = nc.dram_tensor("qm", [128, NT], F32, kind="ExternalInput")
    out = nc.dram_tensor("out", [T, D], F32, kind="ExternalOutput")

    with tile.TileContext(nc) as tc:
        with tc.tile_pool(name="const", bufs=1) as cp:
            # first-needed tensors first: phase-1 kfeat path
            wk_sb = cp.tile([128, ND, S], F8)
            nc.sync.dma_start(wk_sb[:], wkc.ap())
            ones_r = cp.tile([1, 128], BF16)
            nc.sync.dma_start(ones_r[:], onr.ap())
            bk_sb = cp.tile([1, S], BF16)
            nc.sync.dma_start(bk_sb[:], bkr.ap())
            km_sb = cp.tile([128, NT], F32)
            nc.sync.dma_start(km_sb[:], km.ap())
            bv_sb = cp.tile([1, D], BF16)
            nc.sync.dma_start(bv_sb[:], bvr.ap())
            # deferred-load tiles (DMAs issued at t==1 to unblock startup)
            ones2 = cp.tile([128, 2, 2], F8)
            qm_sb = cp.tile([128, NT], F32)
            bq_sb = cp.tile([128, NS], F32)
            bg_sb = cp.tile([128, NF], F32)
            bo_bc = cp.tile([128, D], F32)
            wq_sb = cp.tile([128, ND, S], F8)
            # persistent across phases
            k_nat = cp.tile([128, NT, S], F8)    # relu^2 key feats
            v0_all = cp.tile([128, NT, S], F8)   # v proj, cols :512
            v_e1 = cp.tile([128, NT, S], F8)     # v proj, cols 512:
            kv_sb = cp.tile([128, NS, D], F8)    # kv/32 (post collective)
            ks_sb = cp.tile([128, NS, 2], F8)    # k_sum/8, duplicated cols
            q_sb = cp.tile([128, NS, T], F8)
            u_sb = cp.tile([128, NF, T], BF16)
            uq_f8 = cp.tile([128, NF, T], F8)    # (u * qkv/32) / 2

            with tc.tile_pool(name="dram", bufs=1, space="DRAM") as dram, \
                 tc.tile_pool(name="pf", bufs=1) as pf:
                bounce_in = dram.tile([S, D + 1], BF16)
                bounce_out = dram.tile([S, D + 1], BF16)

                def load_qc(qch):
                    qc = pf.tile([128, ND, 512], F8, name="qc",
                                 tag="qc", bufs=3)
                    nc.sync.dma_start(qc[:], qTc.ap()[qch])
                    return qc
                qc_pre = {}

                # ================= phase 1: k features, v proj, kv =======
                with tc.tile_pool(name="p1", bufs=1) as p1, \
                     tc.tile_pool(name="ps1", bufs=1, space="PSUM") as ps1:
                    wv_sb = p1.tile([128, ND, D], F8)

                    kv0 = [ps1.tile([128, S], F32, name=f"kv0_{s}", tag="kv0",
                                    bufs=NS) for s in range(NS)]

                    for t in range(NT):
                        kc = p1.tile([128, ND, 128], F8, name="kc",
                                     tag="kc", bufs=3)
                        nc.sync.dma_start(kc[:], kTc.ap()[t])
                        vc = p1.tile([128, ND, 128], F8, name="vc",
                                     tag="vc", bufs=3)
                        nc.sync.dma_start(vc[:], vTc.ap()[t])
                        if t == 0:
                            nc.sync.dma_start(wv_sb[:], wvc.ap())
                        if t == 1:
                            qc_pre[0] = load_qc(0)
                            nc.sync.dma_start(wq_sb[:], wqc.ap())
                            nc.sync.dma_start(ones2[:], on2.ap())
                            nc.sync.dma_start(qm_sb[:], qm.ap())
                            nc.sync.dma_start(bq_sb[:], bqc.ap())
                            nc.sync.dma_start(bg_sb[:], bgc.ap())
                            nc.sync.dma_start(bo_bc[:], bor.ap())

                        # k features: relu(K Wk^T + bk)^2 * km -> k_nat[:,t,:]
                        kb = ps1.tile([128, S], F32, name="kb", tag="kb", bufs=2)
                        for c in range(ND2):
                            nc.tensor.matmul(kb[:], kc[:, 2 * c:2 * c + 2, :],
                                             wk_sb[:, 2 * c:2 * c + 2, :],
                                             start=(c == 0), stop=False,
                                             perf_mode=PM.DoubleRow)
                        nc.tensor.matmul(kb[:], ones_r[:], bk_sb[:],
                                         start=False, stop=True)
                        krelu = p1.tile([128, S], F32, name="krelu",
                                        tag="krelu", bufs=2)
                        nc.scalar.activation(krelu[:], kb[:], AF.Relu)
                        nc.vector.scalar_tensor_tensor(
                            k_nat[:, t, :], krelu[:], km_sb[:, t:t + 1], krelu[:],
                            op0=ALU.mult, op1=ALU.mult)

                        # v projection: V Wv^T + bv -> [128, 1024]
                        vb = ps1.tile([128, D], F32, name="vb", tag="vb", bufs=1)
                        for half in range(2):
                            for c in range(ND2):
                                nc.tensor.matmul(
                                    vb[:, half * S:(half + 1) * S],
                                    vc[:, 2 * c:2 * c + 2, :],
                                    wv_sb[:, 2 * c:2 * c + 2,
                                          half * S:(half + 1) * S],
                                    start=(c == 0), stop=False,
                                    perf_mode=PM.DoubleRow)
                            nc.tensor.matmul(
                                vb[:, half * S:(half + 1) * S], ones_r[:],
                                bv_sb[:, half * S:(half + 1) * S],
                                start=False, stop=True)
                        nc.scalar.activation(v0_all[:, t, :], vb[:, 0:S],
                                             AF.Copy)
                        nc.vector.tensor_copy(v_e1[:, t, :], vb[:, S:2 * S])

                        # kv e-half 0 accumulates across t pairs (DoubleRow)
                        if t % 2 == 1:
                            for s in range(NS):
                                nc.tensor.matmul(
                                    kv0[s][:],
                                    k_nat[:, t - 1:t + 1, s * 128:(s + 1) * 128],
                                    v0_all[:, t - 1:t + 1, :],
                                    start=(t == 1), stop=(t == NT - 1),
                                    perf_mode=PM.DoubleRow)

                    for s in range(NS):
                        kvst = p1.tile([128, S], BF16, name="kvst",
                                       tag="kvst", bufs=2)
                        nc.scalar.activation(kvst[:], kv0[s][:], AF.Copy)
                        nc.sync.dma_start(
                            bounce_in[s * 128:(s + 1) * 128, 0:S], kvst[:])

                # kv e-half 1 + k_sum (separate PSUM pool after ps1 frees)
                with tc.tile_pool(name="p1b", bufs=1) as p1b, \
                     tc.tile_pool(name="ps1b", bufs=1, space="PSUM") as ps1b:
                    for s in range(NS):
                        kv1 = ps1b.tile([128, S], F32, name=f"kv1_{s}",
                                        tag="kv1", bufs=2)
                        for tp in range(NT // 2):
                            nc.tensor.matmul(
                                kv1[:],
                                k_nat[:, 2 * tp:2 * tp + 2,
                                      s * 128:(s + 1) * 128],
                                v_e1[:, 2 * tp:2 * tp + 2, :],
                                start=(tp == 0), stop=(tp == NT // 2 - 1),
                                perf_mode=PM.DoubleRow)
                        kvst1 = p1b.tile([128, S], BF16, name="kvst1",
                                         tag="kvst1", bufs=2)
                        nc.scalar.activation(kvst1[:], kv1[:], AF.Copy)
                        nc.sync.dma_start(
                            bounce_in[s * 128:(s + 1) * 128, S:2 * S], kvst1[:])
                        ks = ps1b.tile([128, 2], F32, name=f"ks_{s}",
                                       tag="ks", bufs=2)
                        for t in range(NT):
                            nc.tensor.matmul(
                                ks[:],
                                k_nat[:, t, s * 128:(s + 1) * 128],
                                ones2[:, 0, :], start=(t == 0),
                                stop=(t == NT - 1))
                        ksst = p1b.tile([128, 1], BF16, name="ksst",
                                        tag="ksst", bufs=2)
                        nc.scalar.activation(ksst[:], ks[:, 0:1], AF.Copy)
                        nc.sync.dma_start(
                            bounce_in[s * 128:(s + 1) * 128, D:D + 1], ksst[:])

                with tc.tile_pool(name="p2", bufs=1) as p2, \
                     tc.tile_pool(name="ps2", bufs=1, space="PSUM") as ps2:
                    if use_collective:
                        nc.gpsimd.collective_compute(
                            "AllReduce", ALU.add,
                            replica_groups=[[0, 1], [2, 3], [4, 5], [6, 7]],
                            ins=[bounce_in.opt()], outs=[bounce_out.opt()])
                        kv_src = bounce_out
                    else:
                        kv_src = bounce_in

                    wg_sb = p2.tile([128, ND, D], F8)
                    nc.sync.dma_start(wg_sb[:], wgc.ap())
                    wo_sb = p2.tile([128, ND, D], F8)
                    nc.sync.dma_start(wo_sb[:], woc.ap())

                    # ---- pass A: q features + u gate (no kv dependency) --
                    for qch in range(NQ):
                        t0 = qch * 512
                        qc = qc_pre.pop(qch, None)
                        if qc is None:
                            qc = load_qc(qch)
                        if qch + 1 < NQ and (qch + 1) not in qc_pre:
                            qc_pre[qch + 1] = load_qc(qch + 1)
                        for s in range(NS):
                            qf = ps2.tile([128, 512], F32, name="qf",
                                          tag="mm", bufs=3)
                            for c in range(ND2):
                                nc.tensor.matmul(
                                    qf[:],
                                    wq_sb[:, 2 * c:2 * c + 2,
                                          s * 128:(s + 1) * 128],
                                    qc[:, 2 * c:2 * c + 2, :],
                                    start=(c == 0), stop=(c == ND2 - 1),
                                    perf_mode=PM.DoubleRow)
                            qrelu = p2.tile([128, 512], BF16, name="qrelu",
                                            tag="qrelu", bufs=2)
                            nc.scalar.activation(qrelu[:], qf[:], AF.Relu,
                                                 bias=bq_sb[:, s:s + 1])
                            nc.vector.tensor_mul(q_sb[:, s, t0:t0 + 512],
                                                 qrelu[:], qrelu[:])
                        for f in range(NF):
                            uf = ps2.tile([128, 512], F32, name="uf",
                                          tag="mm", bufs=3)
                            for c in range(ND2):
                                nc.tensor.matmul(
                                    uf[:],
                                    wg_sb[:, 2 * c:2 * c + 2,
                                          f * 128:(f + 1) * 128],
                                    qc[:, 2 * c:2 * c + 2, :],
                                    start=(c == 0), stop=(c == ND2 - 1),
                                    perf_mode=PM.DoubleRow)
                            nc.scalar.activation(u_sb[:, f, t0:t0 + 512], uf[:],
                                                 AF.Silu, bias=bg_sb[:, f:f + 1])

                    # ---- unpack kv + k_sum (after pass A in engine order) -
                    for c in range(NS):
                        kv_f = p2.tile([128, D], BF16, name="kv_f",
                                       tag="kv_f", bufs=2)
                        nc.sync.dma_start(
                            kv_f[:], kv_src[c * 128:(c + 1) * 128, 0:D])
                        nc.vector.tensor_scalar_mul(kv_sb[:, c, :], kv_f[:],
                                                    1.0 / KV_SCL)
                    ks_f = p2.tile([128, NS], BF16)
                    nc.sync.dma_start(
                        ks_f[:],
                        kv_src[:, D:D + 1].rearrange("(c p) o -> p (c o)", p=128))
                    for c in range(NS):
                        for j in range(2):
                            nc.vector.tensor_scalar_mul(
                                ks_sb[:, c, j:j + 1], ks_f[:, c:c + 1],
                                1.0 / KS_SCL)

                    # ---- pass B: qkv, z, gated output projection ---------
                    for qch in range(NQ):
                        t0 = qch * 512
                        for f in range(NF):
                            qk = ps2.tile([128, 512], F32, name="qk",
                                          tag="mm", bufs=3)
                            for c in range(NS2):
                                nc.tensor.matmul(
                                    qk[:],
                                    kv_sb[:, 2 * c:2 * c + 2,
                                          f * 128:(f + 1) * 128],
                                    q_sb[:, 2 * c:2 * c + 2, t0:t0 + 512],
                                    start=(c == 0), stop=(c == NS2 - 1),
                                    perf_mode=PM.DoubleRow)
                            nc.vector.scalar_tensor_tensor(
                                uq_f8[:, f, t0:t0 + 512],
                                u_sb[:, f, t0:t0 + 512], 0.5, qk[:],
                                op0=ALU.mult, op1=ALU.mult)
                        for tt in range(4):
                            ti = qch * 4 + tt
                            zp = ps2.tile([128, 2], F32, name="zp",
                                          tag="z", bufs=2)
                            for c in range(NS):
                                nc.tensor.matmul(
                                    zp[:],
                                    q_sb[:, c, ti * 128:(ti + 1) * 128],
                                    ks_sb[:, c, :],
                                    start=(c == 0), stop=(c == NS - 1))
                            z_sb = p2.tile([128, 1], F32, name="z_sb",
                                           tag="z_sb", bufs=2)
                            nc.vector.tensor_scalar_add(z_sb[:], zp[:, 0:1],
                                                        EPS / KS_SCL)
                            zi = p2.tile([128, 1], F32, name="zi",
                                         tag="zi", bufs=2)
                            nc.vector.reciprocal(zi[:], z_sb[:])
                            # zi *= qm * (KV_SCL/KS_SCL)
                            nc.vector.scalar_tensor_tensor(
                                zi[:], qm_sb[:, ti:ti + 1],
                                2.0 * KV_SCL / KS_SCL,
                                zi[:], op0=ALU.mult, op1=ALU.mult)

                            o_sb = p2.tile([128, D], F32, name="o_sb",
                                           tag="o_sb", bufs=2)
                            for half in range(2):
                                op = ps2.tile([128, 512], F32, name="op",
                                              tag="out", bufs=2)
                                for f2 in range(NF // 2):
                                    nc.tensor.matmul(
                                        op[:],
                                        uq_f8[:, 2 * f2:2 * f2 + 2,
                                              ti * 128:(ti + 1) * 128],
                                        wo_sb[:, 2 * f2:2 * f2 + 2,
                                              half * S:(half + 1) * S],
                                        start=(f2 == 0), stop=(f2 == NF // 2 - 1),
                                        perf_mode=PM.DoubleRow)
                                nc.scalar.activation(
                                    o_sb[:, half * S:(half + 1) * S], op[:],
                                    AF.Copy, scale=zi[:])
                                nc.vector.scalar_tensor_tensor(
                                    o_sb[:, half * S:(half + 1) * S],
                                    bo_bc[:, half * S:(half + 1) * S],
                                    qm_sb[:, ti:ti + 1],
                                    o_sb[:, half * S:(half + 1) * S],
                                    op0=ALU.mult, op1=ALU.add)
                            nc.sync.dma_start(
                                out.ap()[ti * 128:(ti + 1) * 128, :], o_sb[:])

    split_sync_waits(nc)
    return nc


_NC_CACHE = {}


def _get_nc(T, use_collective=True):
    key = (T, use_collective)
    if key not in _NC_CACHE:
        _NC_CACHE[key] = build_nc(T, use_collective)
    return _NC_CACHE[key]


def _chunk_T(xT, chunk):
    # [D, T] -> [T//chunk, 128, ND*chunk]: (t, p, c*chunk+j) = x[c*128+p, t*chunk+j]
    Dd, T = xT.shape
    x = xT.reshape(Dd // 128, 128, T // chunk, chunk)
    return np.ascontiguousarray(x.transpose(2, 1, 0, 3).reshape(
        T // chunk, 128, (Dd // 128) * chunk))


def _chunk_W(wT):
    # [D, F] -> [128, ND*F]: (p, c*F+s) = wT[c*128+p, s]
    Dd, Fd = wT.shape
    return np.ascontiguousarray(
        wT.reshape(Dd // 128, 128, Fd).transpose(1, 0, 2).reshape(
            128, (Dd // 128) * Fd))


def make_in_maps(queries, keys, values, query_mask, key_mask,
                 Wg, bg, Wv, bv, Wq, bq, Wk, bk, Wo, bo):
    B, T_full, _ = queries.shape
    Th = T_full // 2
    NT = Th // 128
    f32 = np.float32
    qTb = np.ascontiguousarray(queries.transpose(0, 2, 1)).astype(NPF8)
    kTb = np.ascontiguousarray(keys.transpose(0, 2, 1)).astype(NPF8)
    vTb = np.ascontiguousarray(values.transpose(0, 2, 1)).astype(NPF8)
    shared = {
        "wgc": _chunk_W(np.ascontiguousarray(Wg.T).astype(NPF8)),
        "wvc": _chunk_W(np.ascontiguousarray(Wv.T).astype(NPF8)),
        "wqc": _chunk_W(np.ascontiguousarray(Wq.T).astype(NPF8)),
        "wkc": _chunk_W(np.ascontiguousarray(Wk.T).astype(NPF8)),
        "woc": _chunk_W(np.ascontiguousarray(Wo.T).astype(NPF8)),
        "bgc": np.ascontiguousarray(
            np.asarray(bg, f32).reshape(D // 128, 128).T),
        "bqc": np.ascontiguousarray(
            np.asarray(bq, f32).reshape(S // 128, 128).T),
        "bkr": np.asarray(bk, f32).reshape(1, S).astype(NPBF),
        "bvr": np.asarray(bv, f32).reshape(1, D).astype(NPBF),
        "bor": np.tile(np.asarray(bo, f32).reshape(1, D), (128, 1)),
        "onr": np.ones((1, 128), NPBF),
        "on2": np.ones((128, 4), NPF8),
    }
    in_maps = []
    for c in range(N_CORES):
        b, h = divmod(c, 2)
        sl = slice(h * Th, (h + 1) * Th)
        m = dict(shared)
        m["qTc"] = _chunk_T(qTb[b][:, sl], 512)
        m["kTc"] = _chunk_T(kTb[b][:, sl], 128)
        m["vTc"] = _chunk_T(vTb[b][:, sl], 128)
        m["km"] = np.ascontiguousarray(
            np.asarray(key_mask[b, sl], f32).reshape(NT, 128).T)
        m["qm"] = np.ascontiguousarray(
            np.asarray(query_mask[b, sl], f32).reshape(NT, 128).T)
        in_maps.append(m)
    return in_maps


def kernel(queries, keys, values, query_mask, key_mask,
           Wg, bg, Wv, bv, Wq, bq, Wk, bk, Wo, bo, _trace=False):
    B, T_full, _ = queries.shape
    Th = T_full // 2
    nc = _get_nc(Th)
    in_maps = make_in_maps(queries, keys, values, query_mask, key_mask,
                           Wg, bg, Wv, bv, Wq, bq, Wk, bk, Wo, bo)
    res = run_bass_kernel_spmd(nc, in_maps, core_ids=list(range(N_CORES)),
                               trace=_trace)
    out = np.empty((B, T_full, D), np.float32)
    for c in range(N_CORES):
        b, h = divmod(c, 2)
        out[b, h * Th:(h + 1) * Th] = res.results[c]["out"]
    if _trace:
        kernel._last_res = res
    return out
